# revision 48
# baseline (speedup 1.0000x reference)
"""Expert-parallel MoE MLP (ExpertMLP) Bass kernel for 8 Trainium2 NeuronCores.

Problem: x[32,4096,256] @ w_fc[32,256,1024] -> gelu(erf) -> @ w_proj[32,1024,256].

Sharding: expert-parallel. Each of the 8 cores gets 4 experts (slices of the
leading axis of every tensor); no cross-core communication. Inside a core, per
expert e:

  1. x[e] ([4096,256], capacity-major) is cast to bf16 (DRAM->DRAM SWDGE cast
     on gpsimd) in 512-row slabs, then each slab is XBar DMA-transposed into
     SBUF as xT [d, c] so the d-contraction of MM1 lies on the partition axis.
     The PE never spends a cycle on transposes.
  2. MM1: hT[h_tile, c_chunk] += w_fc_tile.T @ xT_chunk - w_fc's natural
     [d, h] layout is the stationary operand, so it needs no transpose.
  3. GELU (exact erf form) runs on the ACT engine as the PSUM->SBUF eviction,
     writing bf16 hT tiles.
  4. MM2 uses hT slices as the *stationary* operand and w_proj's natural
     [h, d] layout as the moving operand: out[c_sub, d] += hT_slice.T @
     w_proj_tile. The result lands directly in [capacity, d] orientation, so
     no output transpose is needed.

Scheduling (what makes it fast):
  - Priority-ordered prologue: w_fc[e0] (scalar HWDGE queue) and the cast+
    transpose chain for e0's first slab are enqueued before everything else,
    so MM1 starts ~13us in instead of waiting ~48us for all casts to drain.
  - MM2 of chunk t runs after MM1 of chunk t+1 (one-chunk software pipeline),
    so the ACT-engine GELU of chunk t completes long before MM2 needs it and
    the PE never stalls on the activation.
  - Queue separation: weights go on the Act HWDGE queue, x-casts on the
    gpsimd SWDGE queue, transposes + output stores on the sync HWDGE queue.
    Transposes/casts for expert e+2 are issued inside expert e's loop so
    tile-pool aliasing never head-of-line-blocks the store stream.
"""

import numpy as np
from contextlib import ExitStack

import bass_rust as _br
import concourse.bass as bass
import concourse.tile as tile
from concourse import mybir
from concourse.bass_utils import run_bass_kernel_spmd

E, CAP, D, H = 32, 4096, 256, 1024
N_CORES = 8
E_PER = E // N_CORES  # 4 experts per core
P = 128
F32 = mybir.dt.float32
F32R = mybir.dt.float32r
BF16 = mybir.dt.bfloat16

KD = D // P        # 2 k-tiles in MM1's contraction
KH = H // P        # 8 k-tiles in MM2's contraction
NC_CHUNK = 512     # capacity chunk processed per MM1/MM2 round == slab size
N_CHUNKS = CAP // NC_CHUNK
H_TILES = H // P
HPACK = 2          # h_tiles packed per PSUM tile / GELU call
NS = NC_CHUNK // P
T_CHUNKS = E_PER * N_CHUNKS  # 32 global (expert, chunk) rounds


def _fix_waits(nc):
    """walrus here accepts only one sync wait per instruction; hoist excess
    waits onto standalone EventSemaphore instructions inserted before the
    offender (same engine => same sequencer order)."""
    for fn in nc.m.functions:
        for bb in fn.blocks:
            new = []
            changed = False
            for inst in bb.instructions:
                si = inst.sync_info
                if si is not None and len(si.on_wait) > 1:
                    waits = list(si.on_wait)
                    for w in waits[:-1]:
                        ev = mybir.InstEventSemaphore(
                            name=nc.get_next_instruction_name()
                        )
                        ev.engine = inst.engine
                        ev.sync_info = _br.SyncInfo(on_wait=[w], on_update=[])
                        nc.register_instruction(ev)
                        new.append(ev)
                    inst.sync_info = _br.SyncInfo(
                        on_wait=waits[-1:], on_update=list(si.on_update)
                    )
                    changed = True
                new.append(inst)
            if changed:
                bb.instructions = new


def _build():
    nc = bass.Bass(trn_type="TRN2", target_bir_lowering=False, debug=False)
    x = nc.dram_tensor("x", [E_PER, CAP, D], F32, kind="ExternalInput").ap()
    w_fc = nc.dram_tensor("w_fc", [E_PER, D, H], F32, kind="ExternalInput").ap()
    w_proj = nc.dram_tensor("w_proj", [E_PER, H, D], F32, kind="ExternalInput").ap()
    out = nc.dram_tensor("out", [E_PER, CAP, D], F32, kind="ExternalOutput").ap()
    # bf16 staging copies of x so the XBar DMA-transpose (2-byte dtype only)
    # can build xT without burning TensorE cycles on identity transposes.
    # One DRAM tensor per (expert, cast-chunk); dependency tracking is
    # tensor-granular. Expert 0 casts small chunks first (its early slabs
    # land with minimum latency); everyone else casts 2048-row halves (few,
    # big DMAs keep SWDGE descriptor generation ahead; their flood starts
    # after e0's small casts drain).
    CAST_ROWS = {
        0: [512, 512, 512, 512, 1024, 1024],
        1: [2048, 2048],
        2: [2048, 2048],
        3: [2048, 2048],
    }
    xbf = [
        [
            nc.dram_tensor(f"xbf{e}_{c}", [rows, D], BF16).ap()
            for c, rows in enumerate(CAST_ROWS[e])
        ]
        for e in range(E_PER)
    ]
    # Transposes are uniformly [512, 256] (one per capacity chunk): a single
    # DMA-transpose's data often lands on ONE DMA engine (~20-25 GB/s), so
    # the XBar stream capacity (~25-50 GB/s) barely covers consumption
    # (256KB per 7.3us chunk = 35 GB/s). Two measures:
    #   - 256KB per transpose bounds single-transpose latency at ~10us
    #   - the stream is SPLIT across both HWDGE queues: even/early slabs are
    #     pre-staged on the sync queue, odd slabs (7+) issue in-loop on the
    #     Act queue (wait-free there: their casts and tile buffers are long
    #     ready, so they never head-of-line-block a GELU).

    with tile.TileContext(nc) as tc, ExitStack() as ctx:
        # xT slab ring: ~2.25 experts' worth in flight; ring aliasing gates
        # a later transpose on the MM1 reads of the expert two ahead, which
        # doubles as the pacing mechanism for the whole staging stream.
        xtp = ctx.enter_context(tc.tile_pool(name="xtp", bufs=18))
        wload = ctx.enter_context(tc.tile_pool(name="wload", bufs=2))
        wfc_p = ctx.enter_context(tc.tile_pool(name="wfc", bufs=2))
        wproj_p = ctx.enter_context(tc.tile_pool(name="wproj", bufs=2))
        ht_p = ctx.enter_context(tc.tile_pool(name="ht", bufs=8))
        out_p = ctx.enter_context(tc.tile_pool(name="outp", bufs=5))
        ps_h = ctx.enter_context(tc.tile_pool(name="ps_h", bufs=2, space="PSUM"))
        ps_o = ctx.enter_context(tc.tile_pool(name="ps_o", bufs=4, space="PSUM"))

        def load_weights(e):
            # raw f32 loads on the Act HWDGE queue (kept clear of the cast
            # and transpose streams), cast to bf16 on the idle DVE.
            wfc_raw = wload.tile([P, KD, H], F32, tag="wl")
            nc.scalar.dma_start(wfc_raw[:], w_fc[e].rearrange("(k p) h -> p k h", p=P))
            wfc = wfc_p.tile([P, KD, H], BF16, tag="wfc")
            nc.vector.tensor_copy(wfc[:], wfc_raw[:])
            wproj_raw = wload.tile([P, KH, D], F32, tag="wl")
            nc.scalar.dma_start(
                wproj_raw[:], w_proj[e].rearrange("(k p) d -> p k d", p=P)
            )
            wproj = wproj_p.tile([P, KH, D], BF16, tag="wproj")
            nc.vector.tensor_copy(wproj[:], wproj_raw[:])
            return wfc, wproj

        # per-(expert, chunk): MM1 moving-operand APs, one per k-tile
        xslices = [[None] * N_CHUNKS for _ in range(E_PER)]
        # global slab index -> (xbf buffer, row offset) for deferred (Act
        # queue) transposes
        slab_src = {}

        def issue_cast(e, c):
            """cast one chunk of x[e] to bf16 (gpsimd SWDGE, DRAM->DRAM).
            Both APs flattened to 1D so descriptors cover long contiguous
            runs and the SWDGE ring never fills (a full ring forces a DRAIN
            barrier on ALL outstanding DMAs)."""
            base = sum(CAST_ROWS[e][:c])
            rows = CAST_ROWS[e][c]
            buf = xbf[e][c]
            nc.gpsimd.dma_start(
                buf[:].rearrange("r d -> (r d)"),
                x[e][base:base + rows].rearrange("r d -> (r d)"),
            )
            for lo in range(0, rows, NC_CHUNK):
                slab_src[e * N_CHUNKS + (base + lo) // NC_CHUNK] = (buf, lo)

        def issue_tpose(g, queue):
            """ONE dma_start_transpose per 512-row slab:
            [512, 256] -> [128, KD, 512] (the 3D output folds the d-tile
            index into the partition dim, yielding both k-tiles at once)."""
            buf, lo = slab_src[g]
            e, s = divmod(g, N_CHUNKS)
            xt = xtp.tile([P, KD, NC_CHUNK], BF16, tag="xt", name=f"xt{g}")
            queue.dma_start_transpose(xt[:], buf[lo:lo + NC_CHUNK])
            xslices[e][s] = [xt[:, k, :] for k in range(KD)]

        # ---- prologue: e0's weights, all casts (SWDGE-batch self-paced),
        # and the sync-queue half of the transpose stream: slabs 0-6 plus
        # all even slabs, in consumption order. Odd slabs >= 7 issue
        # in-loop on the Act queue.
        w = [None] * E_PER
        w[0] = load_weights(0)
        for e in range(E_PER):
            for c in range(len(CAST_ROWS[e])):
                issue_cast(e, c)
        for g in range(T_CHUNKS):
            if g < 7 or g % 2 == 0:
                issue_tpose(g, nc.sync)

        # pending MM2 work: (e, nci, ht_tiles) of the previous chunk round
        pend = None

        def run_mm2(p_e, p_nci, p_ht, last):
            wproj_t = w[p_e][1]
            psos = [
                ps_o.tile([P, 2 * D], F32, tag="pso",
                          name=f"pso{p_e}_{p_nci}_{i}")
                for i in range(NS)
            ]
            ob = out_p.tile([P, NS, D], F32, tag="ob")
            order = (
                [(s, k) for s in range(NS) for k in range(KH)]
                if last else
                [(s, k) for k in range(KH) for s in range(NS)]
            )
            for s, k in order:
                nc.tensor.matmul(
                    psos[s][:, :D],
                    p_ht[k // HPACK][:, k % HPACK, s * P:(s + 1) * P],
                    wproj_t[:, k, :],
                    start=(k == 0),
                    stop=(k == KH - 1),
                )
                if last and k == KH - 1:
                    # final round: per-subtile eviction+store so the output
                    # tail overlaps the last matmuls
                    nc.vector.tensor_copy(ob[:, s, :], psos[s][:, :D])
                    nc.gpsimd.dma_start(
                        out[p_e, p_nci * NC_CHUNK + s * P:
                            p_nci * NC_CHUNK + (s + 1) * P, :],
                        ob[:, s, :],
                    )
            if not last:
                for s, pso in enumerate(psos):
                    nc.vector.tensor_copy(ob[:, s, :], pso[:, :D])
                csl = slice(p_nci * NC_CHUNK, (p_nci + 1) * NC_CHUNK)
                nc.gpsimd.dma_start(
                    out[p_e, csl, :].rearrange("(s p) d -> p s d", p=P), ob[:]
                )

        for t in range(T_CHUNKS + 1):
            if t < T_CHUNKS:
                e, nci = divmod(t, N_CHUNKS)
                if nci == 0 and e + 1 < E_PER:
                    w[e + 1] = load_weights(e + 1)
                wfc_t = w[e][0]
                # Act-queue half of the transpose stream: slab t+5 (odd,
                # >=7). Its cast and its ring buffer are long done by now,
                # so the instruction is wait-free on the Act engine and
                # never delays a GELU.
                g = t + 5
                if 7 <= g < T_CHUNKS and g % 2 == 1:
                    issue_tpose(g, nc.sync)
                # ---- MM1 -> GELU for chunk t ----
                # MM1 accumulates HPACK h_tiles into one 2-bank PSUM tile so
                # GELU evicts in wide ACTIVATE calls; hT is written bf16 so
                # MM2's per-matmul weight loads run at 2-byte FWL speed.
                xk = xslices[e][nci]
                ht_tiles = []
                for hp in range(H_TILES // HPACK):
                    psh = ps_h.tile([P, HPACK, NC_CHUNK], F32, tag="psh")
                    for j in range(HPACK):
                        hi = hp * HPACK + j
                        for k in range(KD):
                            nc.tensor.matmul(
                                psh[:, j, :],
                                wfc_t[:, k, hi * P:(hi + 1) * P],
                                xk[k],
                                start=(k == 0),
                                stop=(k == KD - 1),
                            )
                    ht = ht_p.tile([P, HPACK, NC_CHUNK], BF16, tag="ht")
                    nc.scalar.activation(
                        ht[:], psh[:], mybir.ActivationFunctionType.Gelu
                    )
                    ht_tiles.append(ht)

            # ---- MM2 for the previous chunk round (one-chunk delay: its
            # GELUs completed during this round's MM1, so the PE never
            # waits on the ACT engine) ----
            if pend is not None:
                p_e, p_nci, p_ht = pend
                run_mm2(p_e, p_nci, p_ht, last=(t == T_CHUNKS))
            pend = (e, nci, ht_tiles) if t < T_CHUNKS else None

    _fix_waits(nc)
    return nc


_CACHE = {}


def _get_nc():
    if "nc" not in _CACHE:
        _CACHE["nc"] = _build()
    return _CACHE["nc"]


def kernel(x, w_fc, w_proj, trace=False):
    assert x.shape == (E, CAP, D) and w_fc.shape == (E, D, H)
    assert w_proj.shape == (E, H, D)
    nc = _get_nc()
    x = np.ascontiguousarray(x, dtype=np.float32)
    w_fc = np.ascontiguousarray(w_fc, dtype=np.float32)
    w_proj = np.ascontiguousarray(w_proj, dtype=np.float32)
    in_maps = [
        {
            "x": x[i * E_PER:(i + 1) * E_PER],
            "w_fc": w_fc[i * E_PER:(i + 1) * E_PER],
            "w_proj": w_proj[i * E_PER:(i + 1) * E_PER],
        }
        for i in range(N_CORES)
    ]
    res = run_bass_kernel_spmd(nc, in_maps, list(range(N_CORES)), trace=trace)
    out = np.concatenate([r["out"] for r in res.results], axis=0)
    if trace:
        kernel.last_results = res
    return out


# revision 49
# speedup vs baseline: 1.2864x; 1.2864x over previous
"""Expert-parallel MoE MLP (ExpertMLP) Bass kernel for 8 Trainium2 NeuronCores.

Problem: x[32,4096,256] @ w_fc[32,256,1024] -> gelu(erf) -> @ w_proj[32,1024,256].

Sharding: expert-parallel. Each of the 8 cores gets 4 experts (slices of the
leading axis of every tensor); no cross-core communication. Inside a core, per
expert e and 512-row capacity chunk:

  1. x slab loads f32 -> SBUF (plain partition-blocked DMA), is cast to bf16
     on the DVE, and is transposed on the PE (8 pipelined 128x128
     transpose-mode matmuls into one PSUM bank, ~84ns each) into xT blocks so
     the d-contraction of MM1 lies on the partition axis. PE transposes
     measure ~84ns back-to-back, so all of x costs ~21us of PE time - far
     more robust than the XBar DMA transpose (~25-35 GB/s, one queue, jitter
     in the 10-25us range per transfer).
  2. MM1: hT[h_tile, c_chunk] += w_fc_tile.T @ xT_chunk - w_fc's natural
     [d, h] layout is the stationary operand, so it needs no transpose.
  3. GELU (exact erf form) runs on the ACT engine as the PSUM->SBUF
     eviction, writing bf16 hT tiles.
  4. MM2 uses hT slices as the *stationary* operand and w_proj's natural
     [h, d] layout as the moving operand: out[c_sub, d] += hT_slice.T @
     w_proj_tile. The result lands directly in [capacity, d] orientation, so
     no output transpose is needed. It runs one chunk BEHIND MM1 (software
     pipeline), so chunk t's GELUs complete during chunk t+1's MM1 and the
     PE never waits on the ACT engine.

Engine/queue assignment (each in-order stream only carries work that is
ready when it reaches the head of the queue):
  - sync HWDGE: x f32 slab loads only (paced by the xsf tile ring)
  - Act HWDGE: weight loads; ACT engine: GELU evictions
  - DVE: x bf16 casts, transpose-PSUM evictions, weight casts, out evictions
  - gpsimd SWDGE: output stores only
  - PE: transposes (chunk t+2), MM1 (chunk t), MM2 (chunk t-1)
"""

import numpy as np
from contextlib import ExitStack

import bass_rust as _br
import concourse.bass as bass
import concourse.tile as tile
from concourse import mybir
from concourse.bass_utils import run_bass_kernel_spmd
from concourse.masks import make_identity

E, CAP, D, H = 32, 4096, 256, 1024
N_CORES = 8
E_PER = E // N_CORES  # 4 experts per core
P = 128
F32 = mybir.dt.float32
BF16 = mybir.dt.bfloat16

KD = D // P        # 2 k-tiles in MM1's contraction
KH = H // P        # 8 k-tiles in MM2's contraction
NC_CHUNK = 512     # capacity chunk processed per MM1/MM2 round == slab size
N_CHUNKS = CAP // NC_CHUNK
H_TILES = H // P
HPACK = 2          # h_tiles packed per PSUM tile / GELU call
NS = NC_CHUNK // P
NBLK = KD * NS     # 8 transposed 128x128 blocks per slab = one PSUM bank
T_CHUNKS = E_PER * N_CHUNKS  # 32 global (expert, chunk) rounds


def _fix_waits(nc):
    """walrus here accepts only one sync wait per instruction; hoist excess
    waits onto standalone EventSemaphore instructions inserted before the
    offender (same engine => same sequencer order)."""
    for fn in nc.m.functions:
        for bb in fn.blocks:
            new = []
            changed = False
            for inst in bb.instructions:
                si = inst.sync_info
                if si is not None and len(si.on_wait) > 1:
                    waits = list(si.on_wait)
                    for w in waits[:-1]:
                        ev = mybir.InstEventSemaphore(
                            name=nc.get_next_instruction_name()
                        )
                        ev.engine = inst.engine
                        ev.sync_info = _br.SyncInfo(on_wait=[w], on_update=[])
                        nc.register_instruction(ev)
                        new.append(ev)
                    inst.sync_info = _br.SyncInfo(
                        on_wait=waits[-1:], on_update=list(si.on_update)
                    )
                    changed = True
                new.append(inst)
            if changed:
                bb.instructions = new


def _build():
    nc = bass.Bass(trn_type="TRN2", target_bir_lowering=False, debug=False)
    x = nc.dram_tensor("x", [E_PER, CAP, D], F32, kind="ExternalInput").ap()
    w_fc = nc.dram_tensor("w_fc", [E_PER, D, H], F32, kind="ExternalInput").ap()
    w_proj = nc.dram_tensor("w_proj", [E_PER, H, D], F32, kind="ExternalInput").ap()
    out = nc.dram_tensor("out", [E_PER, CAP, D], F32, kind="ExternalOutput").ap()

    with tile.TileContext(nc) as tc, ExitStack() as ctx:
        # x staging rings (per 512-row slab):
        #   xsf: f32 slab off HBM [128, NS, 256] (4 KB/part)
        #   xsb: bf16 copy        [128, NS, 256] (2 KB/part)
        #   xtp: xT blocks        [128, NBLK, 128] (2 KB/part)
        xsf = ctx.enter_context(tc.tile_pool(name="xsf", bufs=12))
        xsb = ctx.enter_context(tc.tile_pool(name="xsb", bufs=4))
        xtp = ctx.enter_context(tc.tile_pool(name="xtp", bufs=4))
        wload = ctx.enter_context(tc.tile_pool(name="wload", bufs=2))
        wfc_p = ctx.enter_context(tc.tile_pool(name="wfc", bufs=2))
        wproj_p = ctx.enter_context(tc.tile_pool(name="wproj", bufs=2))
        idp = ctx.enter_context(tc.tile_pool(name="idp", bufs=1))
        ht_p = ctx.enter_context(tc.tile_pool(name="ht", bufs=8))
        out_p = ctx.enter_context(tc.tile_pool(name="outp", bufs=5))
        # PSUM: ps_h 2x2 banks (MM1+GELU), ps_o 3x1 banks (MM2, two live
        # subtile accumulators + rotation slack), ps_t 1 bank (transposes)
        ps_h = ctx.enter_context(tc.tile_pool(name="ps_h", bufs=2, space="PSUM"))
        ps_o = ctx.enter_context(tc.tile_pool(name="ps_o", bufs=3, space="PSUM"))
        ps_t = ctx.enter_context(tc.tile_pool(name="ps_t", bufs=1, space="PSUM"))

        ident = idp.tile([P, P], BF16, tag="id")
        make_identity(nc, ident[:])

        def load_weights(e):
            # raw f32 loads on the Act HWDGE queue, cast to bf16 on the DVE.
            wfc_raw = wload.tile([P, KD, H], F32, tag="wl")
            nc.scalar.dma_start(wfc_raw[:], w_fc[e].rearrange("(k p) h -> p k h", p=P))
            wfc = wfc_p.tile([P, KD, H], BF16, tag="wfc")
            nc.vector.tensor_copy(wfc[:], wfc_raw[:])
            wproj_raw = wload.tile([P, KH, D], F32, tag="wl")
            nc.scalar.dma_start(
                wproj_raw[:], w_proj[e].rearrange("(k p) d -> p k d", p=P)
            )
            wproj = wproj_p.tile([P, KH, D], BF16, tag="wproj")
            nc.vector.tensor_copy(wproj[:], wproj_raw[:])
            return wfc, wproj

        # per-chunk staging state
        xf_t = [None] * T_CHUNKS   # f32 slab tiles
        xb_t = [None] * T_CHUNKS   # bf16 slab tiles
        xk_t = [None] * T_CHUNKS   # list per k of MM1 moving-operand views

        def stage_load(g):
            e, s = divmod(g, N_CHUNKS)
            rs = slice(s * NC_CHUNK, (s + 1) * NC_CHUNK)
            xf = xsf.tile([P, NS, D], F32, tag="xf", name=f"xf{g}")
            nc.sync.dma_start(
                xf[:], x[e][rs].rearrange("(b p) d -> p b d", p=P)
            )
            xf_t[g] = xf

        def stage_cast(g):
            xb = xsb.tile([P, NS, D], BF16, tag="xb", name=f"xb{g}")
            nc.vector.tensor_copy(xb[:], xf_t[g][:])
            xb_t[g] = xb

        def stage_tpose(g):
            """8 pipelined PE transposes of 128x128 bf16 blocks into one
            PSUM bank (block (k,b) <- x slab rows b, d-cols k), then one DVE
            eviction to SBUF. start=True only on the first block: start
            clears the whole 2KB bank, later blocks land in disjoint
            regions of the zeroed bank via accumulation."""
            pst = ps_t.tile([P, NBLK, P], BF16, tag="pst", name=f"pst{g}")
            xb = xb_t[g]
            first = True
            for k in range(KD):
                for b in range(NS):
                    nc.tensor.transpose(
                        pst[:, k * NS + b, :],
                        xb[:, b, k * P:(k + 1) * P],
                        ident[:],
                    )
                    first = False
            xt = xtp.tile([P, NBLK, P], BF16, tag="xt", name=f"xt{g}")
            nc.vector.tensor_copy(xt[:], pst[:])
            xk_t[g] = [xt[:, k * NS:(k + 1) * NS, :] for k in range(KD)]

        # ---- prologue: expert 0's weights; loads/casts/transposes for the
        # first couple slabs (the rest staged in-loop, staggered).
        w = [None] * E_PER
        w[0] = load_weights(0)
        for g in range(4):
            stage_load(g)
        for g in range(3):
            stage_cast(g)
        for g in range(2):
            stage_tpose(g)

        # pending MM2 work: (e, nci, ht_tiles) of the previous chunk round
        pend = None

        def run_mm2(p_e, p_nci, p_ht, last):
            wproj_t = w[p_e][1]
            ob = out_p.tile([P, NS, D], F32, tag="ob")
            if last:
                # final round: s-major with per-subtile eviction+store so
                # the output tail overlaps the last matmuls
                for s in range(NS):
                    pso = ps_o.tile([P, 2 * D], F32, tag="pso",
                                    name=f"psoL{s}")
                    for k in range(KH):
                        nc.tensor.matmul(
                            pso[:, :D],
                            p_ht[k // HPACK][:, k % HPACK, s * P:(s + 1) * P],
                            wproj_t[:, k, :],
                            start=(k == 0),
                            stop=(k == KH - 1),
                        )
                    nc.vector.tensor_copy(ob[:, s, :], pso[:, :D])
                    nc.gpsimd.dma_start(
                        out[p_e, p_nci * NC_CHUNK + s * P:
                            p_nci * NC_CHUNK + (s + 1) * P, :],
                        ob[:, s, :],
                    )
                return
            # two subtile halves, k-major inside each: only 2 live PSUM
            # accumulators at a time (3-buf ring covers the rotation), which
            # frees the bank the PE transposes need.
            for half in range(2):
                ss = (2 * half, 2 * half + 1)
                psos = {
                    s: ps_o.tile([P, 2 * D], F32, tag="pso",
                                 name=f"pso{p_e}_{p_nci}_{s}")
                    for s in ss
                }
                for k in range(KH):
                    for s in ss:
                        nc.tensor.matmul(
                            psos[s][:, :D],
                            p_ht[k // HPACK][:, k % HPACK, s * P:(s + 1) * P],
                            wproj_t[:, k, :],
                            start=(k == 0),
                            stop=(k == KH - 1),
                        )
                for s in ss:
                    nc.vector.tensor_copy(ob[:, s, :], psos[s][:, :D])
            csl = slice(p_nci * NC_CHUNK, (p_nci + 1) * NC_CHUNK)
            nc.gpsimd.dma_start(
                out[p_e, csl, :].rearrange("(s p) d -> p s d", p=P), ob[:]
            )

        for t in range(T_CHUNKS + 1):
            if t < T_CHUNKS:
                e, nci = divmod(t, N_CHUNKS)
                if nci == 0 and e + 1 < E_PER:
                    w[e + 1] = load_weights(e + 1)
                wfc_t = w[e][0]
                # staggered staging: load slab t+4, cast slab t+3,
                # PE-transpose slab t+2
                if t + 4 < T_CHUNKS:
                    stage_load(t + 4)
                if 3 <= t + 3 < T_CHUNKS:
                    stage_cast(t + 3)
                if 2 <= t + 2 < T_CHUNKS:
                    stage_tpose(t + 2)
                # ---- MM1 -> GELU for chunk t ----
                # MM1 accumulates HPACK h_tiles into one 2-bank PSUM tile so
                # GELU evicts in wide ACTIVATE calls; hT is written bf16 so
                # MM2's per-matmul weight loads run at 2-byte FWL speed.
                xk = xk_t[t]
                ht_tiles = []
                for hp in range(H_TILES // HPACK):
                    psh = ps_h.tile([P, HPACK, NC_CHUNK], F32, tag="psh")
                    for j in range(HPACK):
                        hi = hp * HPACK + j
                        for k in range(KD):
                            nc.tensor.matmul(
                                psh[:, j, :],
                                wfc_t[:, k, hi * P:(hi + 1) * P],
                                xk[k],
                                start=(k == 0),
                                stop=(k == KD - 1),
                            )
                    ht = ht_p.tile([P, HPACK, NC_CHUNK], BF16, tag="ht")
                    nc.scalar.activation(
                        ht[:], psh[:], mybir.ActivationFunctionType.Gelu
                    )
                    ht_tiles.append(ht)
            # ---- MM2 for the previous chunk round (one-chunk delay: its
            # GELUs completed during this round's MM1, so the PE never
            # waits on the ACT engine) ----
            if pend is not None:
                p_e, p_nci, p_ht = pend
                run_mm2(p_e, p_nci, p_ht, last=(t == T_CHUNKS))
            pend = (e, nci, ht_tiles) if t < T_CHUNKS else None

    _fix_waits(nc)
    return nc


_CACHE = {}


def _get_nc():
    if "nc" not in _CACHE:
        _CACHE["nc"] = _build()
    return _CACHE["nc"]


def kernel(x, w_fc, w_proj, trace=False):
    assert x.shape == (E, CAP, D) and w_fc.shape == (E, D, H)
    assert w_proj.shape == (E, H, D)
    nc = _get_nc()
    x = np.ascontiguousarray(x, dtype=np.float32)
    w_fc = np.ascontiguousarray(w_fc, dtype=np.float32)
    w_proj = np.ascontiguousarray(w_proj, dtype=np.float32)
    in_maps = [
        {
            "x": x[i * E_PER:(i + 1) * E_PER],
            "w_fc": w_fc[i * E_PER:(i + 1) * E_PER],
            "w_proj": w_proj[i * E_PER:(i + 1) * E_PER],
        }
        for i in range(N_CORES)
    ]
    res = run_bass_kernel_spmd(nc, in_maps, list(range(N_CORES)), trace=trace)
    out = np.concatenate([r["out"] for r in res.results], axis=0)
    if trace:
        kernel.last_results = res
    return out


# revision 50
# speedup vs baseline: 1.4872x; 1.1561x over previous
"""Expert-parallel MoE MLP (ExpertMLP) Bass kernel for 8 Trainium2 NeuronCores.

Problem: x[32,4096,256] @ w_fc[32,256,1024] -> gelu(erf) -> @ w_proj[32,1024,256].

Sharding: expert-parallel. Each of the 8 cores gets 4 experts (slices of the
leading axis of every tensor); no cross-core communication. Inside a core, per
expert e and 512-row capacity chunk:

  1. x slab loads f32 -> SBUF (plain partition-blocked DMA), is cast to bf16
     on the DVE, and is transposed on the PE (8 pipelined 128x128
     transpose-mode matmuls into one PSUM bank, ~84ns each) into xT blocks so
     the d-contraction of MM1 lies on the partition axis. PE transposes
     measure ~84ns back-to-back, so all of x costs ~21us of PE time - far
     more robust than the XBar DMA transpose (~25-35 GB/s, one queue, jitter
     in the 10-25us range per transfer).
  2. MM1: hT[h_tile, c_chunk] += w_fc_tile.T @ xT_chunk - w_fc's natural
     [d, h] layout is the stationary operand, so it needs no transpose.
  3. GELU (exact erf form) runs on the ACT engine as the PSUM->SBUF
     eviction, writing bf16 hT tiles.
  4. MM2 uses hT slices as the *stationary* operand and w_proj's natural
     [h, d] layout as the moving operand: out[c_sub, d] += hT_slice.T @
     w_proj_tile. The result lands directly in [capacity, d] orientation, so
     no output transpose is needed. It runs one chunk BEHIND MM1 (software
     pipeline), so chunk t's GELUs complete during chunk t+1's MM1 and the
     PE never waits on the ACT engine.

Engine/queue assignment (each in-order stream only carries work that is
ready when it reaches the head of the queue):
  - sync HWDGE: x f32 slab loads only (paced by the xsf tile ring)
  - Act HWDGE: weight loads; ACT engine: GELU evictions
  - DVE: x bf16 casts, transpose-PSUM evictions, weight casts, out evictions
  - gpsimd SWDGE: output stores only
  - PE: transposes (chunk t+2), MM1 (chunk t), MM2 (chunk t-1)
"""

import numpy as np
from contextlib import ExitStack

import bass_rust as _br
import concourse.bass as bass
import concourse.tile as tile
from concourse import mybir
from concourse.bass_utils import run_bass_kernel_spmd
from concourse.masks import make_identity

E, CAP, D, H = 32, 4096, 256, 1024
N_CORES = 8
E_PER = E // N_CORES  # 4 experts per core
P = 128
F32 = mybir.dt.float32
BF16 = mybir.dt.bfloat16

KD = D // P        # 2 k-tiles in MM1's contraction
KH = H // P        # 8 k-tiles in MM2's contraction
NC_CHUNK = 512     # capacity chunk processed per MM1/MM2 round == slab size
N_CHUNKS = CAP // NC_CHUNK
H_TILES = H // P
HPACK = 2          # h_tiles packed per PSUM tile / GELU call
NS = NC_CHUNK // P
NBLK = KD * NS     # 8 transposed 128x128 blocks per slab = one PSUM bank
T_CHUNKS = E_PER * N_CHUNKS  # 32 global (expert, chunk) rounds


def _fix_waits(nc):
    """walrus here accepts only one sync wait per instruction; hoist excess
    waits onto standalone EventSemaphore instructions inserted before the
    offender (same engine => same sequencer order)."""
    for fn in nc.m.functions:
        for bb in fn.blocks:
            new = []
            changed = False
            for inst in bb.instructions:
                si = inst.sync_info
                if si is not None and len(si.on_wait) > 1:
                    waits = list(si.on_wait)
                    for w in waits[:-1]:
                        ev = mybir.InstEventSemaphore(
                            name=nc.get_next_instruction_name()
                        )
                        ev.engine = inst.engine
                        ev.sync_info = _br.SyncInfo(on_wait=[w], on_update=[])
                        nc.register_instruction(ev)
                        new.append(ev)
                    inst.sync_info = _br.SyncInfo(
                        on_wait=waits[-1:], on_update=list(si.on_update)
                    )
                    changed = True
                new.append(inst)
            if changed:
                bb.instructions = new


def _build():
    nc = bass.Bass(trn_type="TRN2", target_bir_lowering=False, debug=False)
    x = nc.dram_tensor("x", [E_PER, CAP, D], F32, kind="ExternalInput").ap()
    w_fc = nc.dram_tensor("w_fc", [E_PER, D, H], F32, kind="ExternalInput").ap()
    w_proj = nc.dram_tensor("w_proj", [E_PER, H, D], F32, kind="ExternalInput").ap()
    out = nc.dram_tensor("out", [E_PER, CAP, D], F32, kind="ExternalOutput").ap()

    with tile.TileContext(nc) as tc, ExitStack() as ctx:
        # x staging rings (per 512-row slab):
        #   xsf: f32 slab off HBM [128, NS, 256] (4 KB/part)
        #   xsb: bf16 copy        [128, NS, 256] (2 KB/part)
        #   xtp: xT blocks        [128, NBLK, 128] (2 KB/part)
        xsf = ctx.enter_context(tc.tile_pool(name="xsf", bufs=12))
        xsb = ctx.enter_context(tc.tile_pool(name="xsb", bufs=4))
        xtp = ctx.enter_context(tc.tile_pool(name="xtp", bufs=4))
        wload = ctx.enter_context(tc.tile_pool(name="wload", bufs=2))
        wfc_p = ctx.enter_context(tc.tile_pool(name="wfc", bufs=2))
        wproj_p = ctx.enter_context(tc.tile_pool(name="wproj", bufs=2))
        idp = ctx.enter_context(tc.tile_pool(name="idp", bufs=1))
        ht_p = ctx.enter_context(tc.tile_pool(name="ht", bufs=8))
        out_p = ctx.enter_context(tc.tile_pool(name="outp", bufs=5))
        # PSUM: ps_h 2x2 banks (MM1+GELU), ps_o 3x1 banks (MM2, two live
        # subtile accumulators + rotation slack), ps_t 1 bank (transposes)
        ps_h = ctx.enter_context(tc.tile_pool(name="ps_h", bufs=2, space="PSUM"))
        ps_o = ctx.enter_context(tc.tile_pool(name="ps_o", bufs=3, space="PSUM"))
        ps_t = ctx.enter_context(tc.tile_pool(name="ps_t", bufs=1, space="PSUM"))

        ident = idp.tile([P, P], BF16, tag="id")
        make_identity(nc, ident[:])

        def load_weights(e):
            # raw f32 loads on the Act HWDGE queue, cast to bf16 on the DVE.
            wfc_raw = wload.tile([P, KD, H], F32, tag="wl")
            nc.scalar.dma_start(wfc_raw[:], w_fc[e].rearrange("(k p) h -> p k h", p=P))
            wfc = wfc_p.tile([P, KD, H], BF16, tag="wfc")
            nc.vector.tensor_copy(wfc[:], wfc_raw[:])
            wproj_raw = wload.tile([P, KH, D], F32, tag="wl")
            nc.scalar.dma_start(
                wproj_raw[:], w_proj[e].rearrange("(k p) d -> p k d", p=P)
            )
            wproj = wproj_p.tile([P, KH, D], BF16, tag="wproj")
            nc.vector.tensor_copy(wproj[:], wproj_raw[:])
            return wfc, wproj

        # per-chunk staging state
        xf_t = [None] * T_CHUNKS   # f32 slab tiles
        xb_t = [None] * T_CHUNKS   # bf16 slab tiles
        xk_t = [None] * T_CHUNKS   # list per k of MM1 moving-operand views

        def stage_load(g):
            e, s = divmod(g, N_CHUNKS)
            rs = slice(s * NC_CHUNK, (s + 1) * NC_CHUNK)
            xf = xsf.tile([P, NS, D], F32, tag="xf", name=f"xf{g}")
            nc.sync.dma_start(
                xf[:], x[e][rs].rearrange("(b p) d -> p b d", p=P)
            )
            xf_t[g] = xf

        def stage_cast(g):
            xb = xsb.tile([P, NS, D], BF16, tag="xb", name=f"xb{g}")
            nc.vector.tensor_copy(xb[:], xf_t[g][:])
            xb_t[g] = xb

        def stage_tpose(g):
            """8 pipelined PE transposes of 128x128 bf16 blocks into one
            PSUM bank (block (k,b) <- x slab rows b, d-cols k), then one DVE
            eviction to SBUF. start=True only on the first block: start
            clears the whole 2KB bank, later blocks land in disjoint
            regions of the zeroed bank via accumulation."""
            pst = ps_t.tile([P, NBLK, P], BF16, tag="pst", name=f"pst{g}")
            xb = xb_t[g]
            first = True
            for k in range(KD):
                for b in range(NS):
                    nc.tensor.transpose(
                        pst[:, k * NS + b, :],
                        xb[:, b, k * P:(k + 1) * P],
                        ident[:],
                    )
                    first = False
            # eviction linearizes blocks (k,b) into [k, 512]-contiguous rows
            # so MM1's moving operand is a plain 2D slice (a strided 3D
            # moving AP costs ~20% matmul throughput)
            xt = xtp.tile([P, KD, NC_CHUNK], BF16, tag="xt", name=f"xt{g}")
            nc.vector.tensor_copy(
                xt[:].rearrange("p k (b c) -> p (k b) c", c=P), pst[:]
            )
            xk_t[g] = [xt[:, k, :] for k in range(KD)]

        # ---- prologue: expert 0's weights; loads/casts/transposes for the
        # first couple slabs (the rest staged in-loop, staggered).
        w = [None] * E_PER
        w[0] = load_weights(0)
        for g in range(4):
            stage_load(g)
        for g in range(3):
            stage_cast(g)
        for g in range(2):
            stage_tpose(g)

        # pending MM2 work: (e, nci, ht_tiles) of the previous chunk round
        pend = None

        def run_mm2(p_e, p_nci, p_ht, last):
            wproj_t = w[p_e][1]
            ob = out_p.tile([P, NS, D], F32, tag="ob")
            if last:
                # final round: s-major with per-subtile eviction+store so
                # the output tail overlaps the last matmuls
                for s in range(NS):
                    pso = ps_o.tile([P, 2 * D], F32, tag="pso",
                                    name=f"psoL{s}")
                    for k in range(KH):
                        nc.tensor.matmul(
                            pso[:, :D],
                            p_ht[k // HPACK][:, k % HPACK, s * P:(s + 1) * P],
                            wproj_t[:, k, :],
                            start=(k == 0),
                            stop=(k == KH - 1),
                        )
                    nc.vector.tensor_copy(ob[:, s, :], pso[:, :D])
                    nc.gpsimd.dma_start(
                        out[p_e, p_nci * NC_CHUNK + s * P:
                            p_nci * NC_CHUNK + (s + 1) * P, :],
                        ob[:, s, :],
                    )
                return
            # two subtile halves, k-major inside each: only 2 live PSUM
            # accumulators at a time (3-buf ring covers the rotation), which
            # frees the bank the PE transposes need.
            for half in range(2):
                ss = (2 * half, 2 * half + 1)
                psos = {
                    s: ps_o.tile([P, 2 * D], F32, tag="pso",
                                 name=f"pso{p_e}_{p_nci}_{s}")
                    for s in ss
                }
                for k in range(KH):
                    for s in ss:
                        nc.tensor.matmul(
                            psos[s][:, :D],
                            p_ht[k // HPACK][:, k % HPACK, s * P:(s + 1) * P],
                            wproj_t[:, k, :],
                            start=(k == 0),
                            stop=(k == KH - 1),
                        )
                for s in ss:
                    nc.vector.tensor_copy(ob[:, s, :], psos[s][:, :D])
            csl = slice(p_nci * NC_CHUNK, (p_nci + 1) * NC_CHUNK)
            nc.gpsimd.dma_start(
                out[p_e, csl, :].rearrange("(s p) d -> p s d", p=P), ob[:]
            )

        for t in range(T_CHUNKS + 1):
            if t < T_CHUNKS:
                e, nci = divmod(t, N_CHUNKS)
                if nci == 0 and e + 1 < E_PER:
                    w[e + 1] = load_weights(e + 1)
                wfc_t = w[e][0]
                # staggered staging: load slab t+4, cast slab t+3,
                # PE-transpose slab t+2
                if t + 4 < T_CHUNKS:
                    stage_load(t + 4)
                if 3 <= t + 3 < T_CHUNKS:
                    stage_cast(t + 3)
                if 2 <= t + 2 < T_CHUNKS:
                    stage_tpose(t + 2)
                # ---- MM1 -> GELU for chunk t ----
                # MM1 accumulates HPACK h_tiles into one 2-bank PSUM tile so
                # GELU evicts in wide ACTIVATE calls; hT is written bf16 so
                # MM2's per-matmul weight loads run at 2-byte FWL speed.
                xk = xk_t[t]
                ht_tiles = []
                for hp in range(H_TILES // HPACK):
                    psh = ps_h.tile([P, HPACK, NC_CHUNK], F32, tag="psh")
                    for j in range(HPACK):
                        hi = hp * HPACK + j
                        for k in range(KD):
                            nc.tensor.matmul(
                                psh[:, j, :],
                                wfc_t[:, k, hi * P:(hi + 1) * P],
                                xk[k],
                                start=(k == 0),
                                stop=(k == KD - 1),
                            )
                    ht = ht_p.tile([P, HPACK, NC_CHUNK], BF16, tag="ht")
                    nc.scalar.activation(
                        ht[:], psh[:], mybir.ActivationFunctionType.Gelu
                    )
                    ht_tiles.append(ht)
            # ---- MM2 for the previous chunk round (one-chunk delay: its
            # GELUs completed during this round's MM1, so the PE never
            # waits on the ACT engine) ----
            if pend is not None:
                p_e, p_nci, p_ht = pend
                run_mm2(p_e, p_nci, p_ht, last=(t == T_CHUNKS))
            pend = (e, nci, ht_tiles) if t < T_CHUNKS else None

    _fix_waits(nc)
    return nc


_CACHE = {}


def _get_nc():
    if "nc" not in _CACHE:
        _CACHE["nc"] = _build()
    return _CACHE["nc"]


def kernel(x, w_fc, w_proj, trace=False):
    assert x.shape == (E, CAP, D) and w_fc.shape == (E, D, H)
    assert w_proj.shape == (E, H, D)
    nc = _get_nc()
    x = np.ascontiguousarray(x, dtype=np.float32)
    w_fc = np.ascontiguousarray(w_fc, dtype=np.float32)
    w_proj = np.ascontiguousarray(w_proj, dtype=np.float32)
    in_maps = [
        {
            "x": x[i * E_PER:(i + 1) * E_PER],
            "w_fc": w_fc[i * E_PER:(i + 1) * E_PER],
            "w_proj": w_proj[i * E_PER:(i + 1) * E_PER],
        }
        for i in range(N_CORES)
    ]
    res = run_bass_kernel_spmd(nc, in_maps, list(range(N_CORES)), trace=trace)
    out = np.concatenate([r["out"] for r in res.results], axis=0)
    if trace:
        kernel.last_results = res
    return out
